# revision 5
# baseline (speedup 1.0000x reference)
"""BalancedMoE (B=8192, D=2048, E=8, top-2) on 8 Trainium2 NeuronCores.

Strategy: expert-parallel with host-side sparse dispatch.
  - Host computes gate logits / top-2 routing / softmax gates, gathers each
    expert's tokens into a [KT, P, C] bf16 layout (full-width k-slices so
    every DMA moves ~4.5KB contiguous runs per partition), and pre-permutes
    the expert weight into [MT, P, KT, P] bf16 m-chunks (4KB runs).
  - Core e runs a dense [D, D] x [D, C] matmul for expert e (top-2 of 8
    experts => 4x less FLOPs than the dense reference). bf16 runs the PE at
    the same 1 column/cycle as fp32r but halves the input DMA footprint, so
    the whole working set (weights 64KB/part + tokens ~70KB/part) stays
    SBUF-resident.
  - Startup schedule: tokens stream on the Sync HWDGE queue, weights on the
    Activation queue (the two queues split the ~360GB/s evenly, arrivals
    auto-interleave).  The first NQ m-rows are processed in k-QUARTERS
    (separate PSUM accumulation groups combined via SBUF partials) so the
    in-order PE queue chases the token k-front instead of head-of-line
    blocking on un-arrived slices.  Remaining rows run full-k accumulation
    with all operands resident.
  - Outputs (f32) drain on the Vector queue; host scatters/combines with the
    gate weights.

Per-core Bass kernel: outT[o, t] = sum_d W_e[o, d] * toks[t, d] + b_e[o]
"""

import os

import numpy as np

P = 128
B = 8192
D_LAT = 1024
D_EMB = 1024
D = D_LAT + D_EMB  # 2048
E = 8
TOPK = 2
N_CORES = 8
KT = D // P  # 16
MT = D // P  # 16
NQ = 4  # number of startup m-rows processed in k-quarters


# ----------------------------------------------------------------- device ---

_cache = {}


def _ntff_shim():
    """Register the axon NTFF profile hook that the boot skips when
    antenv.axon_hooks is missing (so BASS_TRACE=1 yields exec_time_ns)."""
    import sys
    import types

    if "antenv.axon_hooks" in sys.modules:
        return
    holder = [None]
    mod = types.ModuleType("antenv.axon_hooks")
    mod.set_axon_ntff_profile_hook = lambda h: holder.__setitem__(0, h)
    mod.get_axon_ntff_profile_hook = lambda: holder[0]
    sys.modules["antenv.axon_hooks"] = mod
    try:
        import antenv

        antenv.axon_hooks = mod
        from trn_agent_boot.trn_boot import _ntff_profile_via_ctypes

        mod.set_axon_ntff_profile_hook(
            _ntff_profile_via_ctypes("/opt/axon/libaxon_pjrt.so")
        )
    except Exception:
        pass


def _n_tiles(C):
    """Split C into moving-operand tiles of width 256..512 (>=256 columns per
    matmul keeps the PE at full rate; PSUM caps a tile at 512)."""
    assert C >= 512
    k = (C - 256) // 512 if C % 512 else C // 512
    rem = C - 512 * k
    sizes = [512] * k
    if rem == 0:
        pass
    elif rem <= 512:
        sizes.append(rem)
    else:  # 513..767: two tiles, both >= 256
        sizes.extend([rem - 256, 256])
    return sizes


def _build(C):
    import concourse.mybir as mybir
    from concourse import bacc
    from concourse.bass import ds
    from concourse.tile import TileContext

    dt = mybir.dt.bfloat16
    f32 = mybir.dt.float32
    n_sizes = _n_tiles(C)
    J = len(n_sizes)
    n_offs = [0] * J
    for j in range(1, J):
        n_offs[j] = n_offs[j - 1] + n_sizes[j - 1]

    nc = bacc.Bacc(
        "TRN2", target_bir_lowering=False, debug=False, num_devices=N_CORES
    )
    # wp[m, ki, ko, o] = W_e[m*128 + o, ko*128 + ki] — per-m chunks are
    # contiguous (4KB/partition) so weight DMAs stay descriptor-efficient.
    wp = nc.dram_tensor("wp", [MT, P, KT, P], dt, kind="ExternalInput")
    # tokk[k, ki, c] = inp[idx[c], k*128 + ki] — full-width k-slices,
    # ~4.5KB contiguous per partition per DMA.
    tokk = nc.dram_tensor("tokk", [KT, P, C], dt, kind="ExternalInput")
    bias = nc.dram_tensor("bias", [D], f32, kind="ExternalInput")
    outT = nc.dram_tensor("outT", [MT, P, C], f32, kind="ExternalOutput")

    b_r = bias.ap().rearrange("(mo mi) -> mi mo", mi=P)

    nq = min(NQ, MT)

    with TileContext(nc) as tc:
        with (
            tc.tile_pool(name="w", bufs=1) as w_pool,
            tc.tile_pool(name="tok", bufs=1) as tok_pool,
            tc.tile_pool(name="acc", bufs=1) as acc_pool,
            tc.tile_pool(name="out", bufs=6) as out_pool,
            tc.tile_pool(name="bias", bufs=1) as b_pool,
            tc.tile_pool(name="ps", bufs=8, space="PSUM") as ps_pool,
        ):
            bias_tile = b_pool.tile([P, MT], f32)
            nc.gpsimd.dma_start(bias_tile[:], b_r)

            # ---- input DMAs: tokens on Sync queue, weights on Activation
            # queue.  Each queue gets ~half the 16 DMA engines' bandwidth, so
            # slice k lands ~(k+1)*3.2us after start, chunk m ~(m+1)*3.0us.
            tok_tiles = []
            for k in range(KT):
                t = tok_pool.tile([P, C], dt, tag=f"t{k}")
                nc.sync.dma_start(t[:], tokk.ap()[k])
                tok_tiles.append(t)
            w_tiles = []
            for m in range(MT):
                w = w_pool.tile([P, KT, P], dt, tag=f"w{m}")
                nc.scalar.dma_start(w[:], wp.ap()[m])
                w_tiles.append(w)

            def rhs(k, j):
                return tok_tiles[k][:, ds(n_offs[j], n_sizes[j])]

            # ---- startup: first nq rows in k-quarters so the PE chases the
            # token arrival front (PSUM partials combined in SBUF).
            acc_tiles = {}
            for q in range(4):
                k0, k1 = q * 4, q * 4 + 4
                for m in range(nq):
                    pss = []
                    for j in range(J):
                        psf = ps_pool.tile([P, 512], f32, tag="ps")
                        pss.append(psf[:, : n_sizes[j]])
                    for k in range(k0, k1):
                        for j in range(J):
                            nc.tensor.matmul(
                                pss[j],
                                w_tiles[m][:, k, :],
                                rhs(k, j),
                                start=(k == k0),
                                stop=(k == k1 - 1),
                            )
                    for j in range(J):
                        if q == 0:
                            a_full = acc_pool.tile(
                                [P, 512], f32, tag=f"acc{m}_{j}"
                            )
                            a = a_full[:, : n_sizes[j]]
                            acc_tiles[(m, j)] = a
                            # ACT engine: a = ps + bias (gpsimd can't read PSUM)
                            nc.scalar.add(a, pss[j], bias_tile[:, m : m + 1])
                        elif q < 3:
                            a = acc_tiles[(m, j)]
                            nc.vector.tensor_add(a, a, pss[j])
                        else:
                            a = acc_tiles[(m, j)]
                            nc.vector.tensor_add(a, a, pss[j])
                            nc.sync.dma_start(
                                outT.ap()[m][:, ds(n_offs[j], n_sizes[j])], a
                            )

            # ---- steady state: full-k rows, everything SBUF-resident.
            for m in range(nq, MT):
                pss = []
                for j in range(J):
                    psf = ps_pool.tile([P, 512], f32, tag="ps")
                    pss.append(psf[:, : n_sizes[j]])
                for k in range(KT):
                    for j in range(J):
                        nc.tensor.matmul(
                            pss[j],
                            w_tiles[m][:, k, :],
                            rhs(k, j),
                            start=(k == 0),
                            stop=(k == KT - 1),
                        )
                for j in range(J):
                    o_full = out_pool.tile([P, 512], f32, tag="out")
                    o = o_full[:, : n_sizes[j]]
                    nc.vector.tensor_scalar_add(
                        o, pss[j], bias_tile[:, m : m + 1]
                    )
                    nc.sync.dma_start(
                        outT.ap()[m][:, ds(n_offs[j], n_sizes[j])], o
                    )
    nc.compile()
    return nc


def _get_program(C):
    if C not in _cache:
        _cache[C] = _build(C)
    return _cache[C]


# ------------------------------------------------------------------- host ---


def kernel(x, y, W_experts, b_experts, W_gate, b_gate):
    import ml_dtypes

    bf16 = np.dtype(ml_dtypes.bfloat16)

    x = np.asarray(x, dtype=np.float32)
    y = np.asarray(y, dtype=np.float32)
    W_experts = np.asarray(W_experts, dtype=np.float32)
    b_experts = np.asarray(b_experts, dtype=np.float32)
    W_gate = np.asarray(W_gate, dtype=np.float32)
    b_gate = np.asarray(b_gate, dtype=np.float32)

    inp = np.concatenate([x, y], axis=1)  # [B, D]

    # ---- routing (host) ----
    logits = inp.astype(np.float64) @ W_gate.T.astype(np.float64) + b_gate
    order = np.argsort(-logits, axis=1, kind="stable")
    top2 = order[:, :TOPK]  # [B, 2]
    v = np.take_along_axis(logits, top2, axis=1)
    v = v - v.max(axis=1, keepdims=True)
    ev = np.exp(v)
    g = (ev / ev.sum(axis=1, keepdims=True)).astype(np.float32)  # [B, 2]

    counts = np.bincount(top2.ravel(), minlength=E)
    C = max(512, int(counts.max()))

    idx_list = []
    wgt_list = []
    for e in range(E):
        m0 = top2[:, 0] == e
        m1 = top2[:, 1] == e
        idx_e = np.concatenate([np.nonzero(m0)[0], np.nonzero(m1)[0]])
        w_e = np.concatenate([g[m0, 0], g[m1, 1]])
        idx_list.append(idx_e)
        wgt_list.append(w_e)

    inp_bf = inp.astype(bf16)  # [B, D]
    in_maps = []
    for e in range(E):
        n_e = len(idx_list[e])
        # tokk[k, p, c] = inp[idx[c], k*128 + p]
        tokk = np.zeros((KT, P, C), dtype=bf16)
        tokk[:, :, :n_e] = inp_bf[idx_list[e]].T.reshape(KT, P, n_e)
        # wp[m, ki, ko, o] = W_e[m*128 + o, ko*128 + ki]
        wpe = np.ascontiguousarray(
            W_experts[e].reshape(MT, P, KT, P).transpose(0, 3, 2, 1).astype(bf16)
        )
        in_maps.append({"wp": wpe, "tokk": tokk, "bias": b_experts[e]})

    # ---- device ----
    if os.environ.get("BASS_TRACE"):
        _ntff_shim()
    from concourse.bass_utils import run_bass_kernel_spmd

    nc = _get_program(C)
    res = None
    for attempt in range(3):
        try:
            res = run_bass_kernel_spmd(nc, in_maps, core_ids=list(range(N_CORES)))
            break
        except Exception:
            # the axon-tunneled device occasionally reports a transient
            # NRT_EXEC_UNIT_UNRECOVERABLE; it recovers after a short wait
            if attempt == 2:
                raise
            import time

            time.sleep(20 * (attempt + 1))
            try:
                import jax

                jax.clear_caches()
            except Exception:
                pass
    globals()["_last_res"] = res
    if res.exec_time_ns is not None:
        print(f"HW exec time: {res.exec_time_ns} ns")

    # ---- combine (host) ----
    fused = np.zeros((B, D), dtype=np.float32)
    for e in range(E):
        n_e = len(idx_list[e])
        if n_e == 0:
            continue
        out_rows = res.results[e]["outT"].reshape(D, C)[:, :n_e].T  # [n_e, D]
        fused[idx_list[e]] += out_rows * wgt_list[e][:, None]
    return fused


# revision 8
# speedup vs baseline: 1.2799x; 1.2799x over previous
"""BalancedMoE (B=8192, D=2048, E=8, top-2) on 8 Trainium2 NeuronCores.

Strategy: expert-parallel with host-side sparse dispatch.
  - Host computes gate logits / top-2 routing / softmax gates, gathers each
    expert's tokens into a partition-major [P, KT, C] bf16 layout, and
    pre-permutes the expert weight into [P, MT, KT, P] bf16.
  - Core e runs a dense [D, D] x [D, C] matmul for expert e (top-2 of 8
    experts => 4x less FLOPs than the dense reference). bf16 runs the PE at
    the same 1 column/cycle as fp32r but halves the input DMA footprint, so
    the whole working set stays SBUF-resident.
  - DMA: per-descriptor overhead (~300ns) dominates small transfers, so every
    DMA moves multi-KB contiguous runs per partition: weights ride the
    Activation queue in 5 grouped chunks (4..16KB runs), tokens ride the Sync
    queue as 2 pair + 3 quad k-slices (9..18KB runs), outputs (bf16) ride the
    GpSimd SWDGE queue as per-row halves (~4.5KB runs).
  - Startup: the first rows are processed in k-eighths/quarters (separate
    PSUM accumulation groups combined via f32 partials in SBUF) so the
    in-order PE queue chases the token k-front instead of head-of-line
    blocking; remaining rows run full-k accumulation (j-outer, k-inner:
    back-to-back same-bank accumulate keeps the PE pipeline full).

Per-core Bass kernel: outT[o, t] = sum_d W_e[o, d] * toks[t, d] + b_e[o]
"""

import os

import numpy as np

P = 128
B = 8192
D_LAT = 1024
D_EMB = 1024
D = D_LAT + D_EMB  # 2048
E = 8
TOPK = 2
N_CORES = 8
KT = D // P  # 16
MT = D // P  # 16
NQ = 3  # startup m-rows processed in k-quarters (m0 in k-eighths for k0..3)

W_GROUPS = [(0, 1), (1, 4), (4, 8), (8, 12), (12, 16)]  # m-chunk DMA groups
T_GROUPS = [(0, 2), (2, 4), (4, 8), (8, 12), (12, 16)]  # k-slice DMA groups


# ----------------------------------------------------------------- device ---

_cache = {}


def _ntff_shim():
    """Register the axon NTFF profile hook that the boot skips when
    antenv.axon_hooks is missing (so BASS_TRACE=1 yields exec_time_ns)."""
    import sys
    import types

    if "antenv.axon_hooks" in sys.modules:
        return
    holder = [None]
    mod = types.ModuleType("antenv.axon_hooks")
    mod.set_axon_ntff_profile_hook = lambda h: holder.__setitem__(0, h)
    mod.get_axon_ntff_profile_hook = lambda: holder[0]
    sys.modules["antenv.axon_hooks"] = mod
    try:
        import antenv

        antenv.axon_hooks = mod
        from trn_agent_boot.trn_boot import _ntff_profile_via_ctypes

        mod.set_axon_ntff_profile_hook(
            _ntff_profile_via_ctypes("/opt/axon/libaxon_pjrt.so")
        )
    except Exception:
        pass


def _n_tiles(C):
    """Split C into moving-operand tiles of width 256..512 (>=256 columns per
    matmul keeps the PE at full rate; PSUM caps a tile at 512)."""
    assert C >= 512
    k = (C - 256) // 512 if C % 512 else C // 512
    rem = C - 512 * k
    sizes = [512] * k
    if rem == 0:
        pass
    elif rem <= 512:
        sizes.append(rem)
    else:  # 513..767: two tiles, both >= 256
        sizes.extend([rem - 256, 256])
    return sizes


def _build(C):
    import concourse.mybir as mybir
    from concourse import bacc
    from concourse.bass import ds
    from concourse.tile import TileContext

    dt = mybir.dt.bfloat16
    f32 = mybir.dt.float32
    n_sizes = _n_tiles(C)
    J = len(n_sizes)
    n_offs = [0] * J
    for j in range(1, J):
        n_offs[j] = n_offs[j - 1] + n_sizes[j - 1]

    nc = bacc.Bacc(
        "TRN2", target_bir_lowering=False, debug=False, num_devices=N_CORES
    )
    # wpt[ki, m, ko, o] = W_e[m*128 + o, ko*128 + ki] — partition-major so a
    # group of m-chunks is one DMA with mg*4KB contiguous runs per partition.
    wpt = nc.dram_tensor("wpt", [P, MT, KT, P], dt, kind="ExternalInput")
    # tokq[ki, k, c] = inp[idx[c], k*128 + ki] — partition-major so a group of
    # k-slices is one DMA with ng*C*2 contiguous runs per partition.
    tokq = nc.dram_tensor("tokq", [P, KT, C], dt, kind="ExternalInput")
    bias = nc.dram_tensor("bias", [D], f32, kind="ExternalInput")
    outT = nc.dram_tensor("outT", [MT, P, C], dt, kind="ExternalOutput")

    b_r = bias.ap().rearrange("(mo mi) -> mi mo", mi=P)
    nq = min(NQ, MT)

    with TileContext(nc) as tc:
        with (
            tc.tile_pool(name="w", bufs=1) as w_pool,
            tc.tile_pool(name="tok", bufs=1) as tok_pool,
            tc.tile_pool(name="acc", bufs=1) as acc_pool,
            tc.tile_pool(name="orow", bufs=6) as orow_pool,
            tc.tile_pool(name="bias", bufs=1) as b_pool,
            tc.tile_pool(name="ps", bufs=8, space="PSUM") as ps_pool,
        ):
            bias_tile = b_pool.tile([P, MT], f32)
            nc.scalar.dma_start(bias_tile[:], b_r)

            # ---- input DMAs: tokens on Sync queue, weights on Activation.
            tok_tiles = []  # one tile per T_GROUP
            for lo, hi in T_GROUPS:
                t = tok_pool.tile([P, hi - lo, C], dt, tag=f"t{lo}")
                nc.sync.dma_start(t[:], tokq.ap()[:, ds(lo, hi - lo)])
                tok_tiles.append(t)
            w_tiles = []  # one tile per W_GROUP
            for lo, hi in W_GROUPS:
                w = w_pool.tile([P, hi - lo, KT, P], dt, tag=f"w{lo}")
                nc.scalar.dma_start(w[:], wpt.ap()[:, ds(lo, hi - lo)])
                w_tiles.append(w)

            def rhs(k, j):
                for gi, (lo, hi) in enumerate(T_GROUPS):
                    if k < hi:
                        return tok_tiles[gi][:, k - lo, ds(n_offs[j], n_sizes[j])]
                raise AssertionError

            def lhsT(m, k):
                for gi, (lo, hi) in enumerate(W_GROUPS):
                    if m < hi:
                        return w_tiles[gi][:, m - lo, k, :]
                raise AssertionError

            def out_row_dmas(m, orow):
                # per-row output in halves so the second half can fly while
                # the tail drains; bf16 keeps runs ~4.5KB
                h = min(1024, C)
                nc.gpsimd.dma_start(outT.ap()[m][:, ds(0, h)], orow[:, ds(0, h)])
                if C > h:
                    nc.gpsimd.dma_start(
                        outT.ap()[m][:, ds(h, C - h)], orow[:, ds(h, C - h)]
                    )

            # ---- startup: first nq rows in k-chunks chasing the token front.
            # m0 row: k-chunks [0,1],[2,3],[4..7],[8..11],[12..15]
            # m1/m2  : k-chunks [0..3],[4..7],[8..11],[12..15]
            chunks = {0: T_GROUPS}
            for m in range(1, nq):
                chunks[m] = [(0, 4), (4, 8), (8, 12), (12, 16)]
            n_chunks = {m: len(chunks[m]) for m in chunks}
            acc_tiles = {}
            orow_q = {}
            # arrival-chasing order: a row emits its k-chunk in the phase of
            # the token group that completes that chunk's k-range
            emit = []  # (m, chunk_index)
            for ci, (tlo, thi) in enumerate(T_GROUPS):
                for m in range(nq):
                    for qi, (klo, khi) in enumerate(chunks[m]):
                        if tlo < khi <= thi:
                            emit.append((m, qi))
            for m, qi in emit:
                klo, khi = chunks[m][qi]
                last = qi == n_chunks[m] - 1
                for j in range(J):
                    psf = ps_pool.tile([P, 512], f32, tag="ps")
                    pj = psf[:, : n_sizes[j]]
                    for k in range(klo, khi):
                        nc.tensor.matmul(
                            pj,
                            lhsT(m, k),
                            rhs(k, j),
                            start=(k == klo),
                            stop=(k == khi - 1),
                        )
                    if qi == 0:
                        a_full = acc_pool.tile([P, 512], f32, tag=f"acc{m}_{j}")
                        a = a_full[:, : n_sizes[j]]
                        acc_tiles[(m, j)] = a
                        # ACT engine: a = ps + bias (keeps DVE free)
                        nc.scalar.add(a, pj, bias_tile[:, m : m + 1])
                    elif not last:
                        a = acc_tiles[(m, j)]
                        nc.vector.tensor_add(a, a, pj)
                    else:
                        if m not in orow_q:
                            orow_q[m] = orow_pool.tile(
                                [P, C], dt, tag="orow", name=f"orow_q{m}"
                            )
                        o = orow_q[m][:, ds(n_offs[j], n_sizes[j])]
                        a = acc_tiles[(m, j)]
                        nc.vector.tensor_add(o, a, pj)
                if last:
                    out_row_dmas(m, orow_q[m])

            # ---- steady state: full-k rows, j-outer k-inner (back-to-back
            # same-bank accumulate keeps the PE pipeline full).
            for m in range(nq, MT):
                orow = orow_pool.tile([P, C], dt, tag="orow")
                for j in range(J):
                    psf = ps_pool.tile([P, 512], f32, tag="ps")
                    pj = psf[:, : n_sizes[j]]
                    for k in range(KT):
                        nc.tensor.matmul(
                            pj,
                            lhsT(m, k),
                            rhs(k, j),
                            start=(k == 0),
                            stop=(k == KT - 1),
                        )
                    o = orow[:, ds(n_offs[j], n_sizes[j])]
                    nc.vector.tensor_scalar_add(o, pj, bias_tile[:, m : m + 1])
                out_row_dmas(m, orow)
    nc.compile()
    return nc


def _get_program(C):
    if C not in _cache:
        _cache[C] = _build(C)
    return _cache[C]


# ------------------------------------------------------------------- host ---


def kernel(x, y, W_experts, b_experts, W_gate, b_gate):
    import ml_dtypes

    bf16 = np.dtype(ml_dtypes.bfloat16)

    x = np.asarray(x, dtype=np.float32)
    y = np.asarray(y, dtype=np.float32)
    W_experts = np.asarray(W_experts, dtype=np.float32)
    b_experts = np.asarray(b_experts, dtype=np.float32)
    W_gate = np.asarray(W_gate, dtype=np.float32)
    b_gate = np.asarray(b_gate, dtype=np.float32)

    inp = np.concatenate([x, y], axis=1)  # [B, D]

    # ---- routing (host) ----
    logits = inp.astype(np.float64) @ W_gate.T.astype(np.float64) + b_gate
    order = np.argsort(-logits, axis=1, kind="stable")
    top2 = order[:, :TOPK]  # [B, 2]
    v = np.take_along_axis(logits, top2, axis=1)
    v = v - v.max(axis=1, keepdims=True)
    ev = np.exp(v)
    g = (ev / ev.sum(axis=1, keepdims=True)).astype(np.float32)  # [B, 2]

    counts = np.bincount(top2.ravel(), minlength=E)
    C = max(512, int(counts.max()))

    idx_list = []
    wgt_list = []
    for e in range(E):
        m0 = top2[:, 0] == e
        m1 = top2[:, 1] == e
        idx_e = np.concatenate([np.nonzero(m0)[0], np.nonzero(m1)[0]])
        w_e = np.concatenate([g[m0, 0], g[m1, 1]])
        idx_list.append(idx_e)
        wgt_list.append(w_e)

    inp_bf = inp.astype(bf16)  # [B, D]
    in_maps = []
    for e in range(E):
        n_e = len(idx_list[e])
        # tokq[p, k, c] = inp[idx[c], k*128 + p]
        tokq = np.zeros((P, KT, C), dtype=bf16)
        tokq[:, :, :n_e] = (
            inp_bf[idx_list[e]].T.reshape(KT, P, n_e).transpose(1, 0, 2)
        )
        # wpt[ki, m, ko, o] = W_e[m*128 + o, ko*128 + ki]
        wpt = np.ascontiguousarray(
            W_experts[e].reshape(MT, P, KT, P).transpose(3, 0, 2, 1).astype(bf16)
        )
        in_maps.append({"wpt": wpt, "tokq": tokq, "bias": b_experts[e]})

    # ---- device ----
    if os.environ.get("BASS_TRACE"):
        _ntff_shim()
    from concourse.bass_utils import run_bass_kernel_spmd

    nc = _get_program(C)
    res = None
    for attempt in range(3):
        try:
            res = run_bass_kernel_spmd(nc, in_maps, core_ids=list(range(N_CORES)))
            break
        except Exception:
            # the axon-tunneled device occasionally reports a transient
            # NRT_EXEC_UNIT_UNRECOVERABLE; it recovers after a short wait
            if attempt == 2:
                raise
            import time

            time.sleep(20 * (attempt + 1))
            try:
                import jax

                jax.clear_caches()
            except Exception:
                pass
    globals()["_last_res"] = res
    if res.exec_time_ns is not None:
        print(f"HW exec time: {res.exec_time_ns} ns")

    # ---- combine (host) ----
    fused = np.zeros((B, D), dtype=np.float32)
    for e in range(E):
        n_e = len(idx_list[e])
        if n_e == 0:
            continue
        out_rows = (
            res.results[e]["outT"].reshape(D, C)[:, :n_e].T.astype(np.float32)
        )
        fused[idx_list[e]] += out_rows * wgt_list[e][:, None]
    return fused
